# revision 16
# baseline (speedup 1.0000x reference)
"""Trainium2 Bass kernel for nn_HKANGNN (hetero GraphConv + KAN head).

Math (only the email-node output path matters):
  e    = x_email @ w_email.T + b_email
  agg_se[n] = sum_{se edges -> n} (x_sender[src] @ w_sender.T + b_sender)
  agg_ue[n] = sum_{ue edges -> n} (x_url[src]    @ w_url.T    + b_url)
  out_e = agg_se @ w_rel_se.T + b_rel_se + agg_ue @ w_rel_ue.T + b_rel_ue
        + e @ (w_root_se + w_root_ue).T
  h = relu(out_e);  out = silu(h) @ base_w.T + einsum(b_splines(h), spline_w)

Device strategy (8 cores, email nodes sharded 12500/core, padded to 13312):
  * linearity folds the tiny projections into a single [34,128] mc_all matmul
    over per-dst aggregates of RAW features + counts (all fp16).
  * per-core node permutation (serpentine by sender-degree over 104 dst
    tiles) balances per-tile edge counts: sender class needs 1 gather group
    per tile, urls 2 -> 66560 gather slots (vs 89600 naive).
  * segment-sum = one-hot matmuls accumulated in a [*,512] PSUM covering 4
    dst tiles; per-edge rows via dma_gather on 4 SWDGE queues.
  * projection: out_e.T[h, n] accumulated in PSUM over 6 K-chunks of
    (Wrootsum@w_email).T as stationary weights (fp16).
  * KAN head with x = clamp(h,0,2.2): spline(h) = poly3(x) [f32 chunks]
    + sum_k w_k * cube_k where cube_k = relu(t_k-x)^3 for t_k<=1.0 (left
    cubes; the (x-t)^3 part folds into the poly) and relu(x-t_k)^3 for
    t_k>1.0.  The cube/rsil chunks are small-valued -> fp16-safe (2x DVE),
    only the monomials stay f32.  9 chunks run 3-way column-tiled on the
    PE; host sums the 3 output strips + q0 during unsharding.
"""

import os
import numpy as np

import concourse.bass as bass
import concourse.mybir as mybir
import concourse.tile as tile
from concourse import bacc
from concourse.bass_utils import run_bass_kernel_spmd

F32 = mybir.dt.float32
F16 = mybir.dt.float16
NF16 = np.float16

N_CORES = 8
HID = 128
NE, NS, NU = 100000, 30000, 50000
NSH = NE // N_CORES          # 12500 real nodes per core
NP = 13312                   # padded (26 x 512 node tiles, 104 x 128 dst tiles)
NT128 = NP // 128            # 104 dst tiles
NT512 = NP // 512            # 26 node tiles
KIN = 768
NKC = KIN // 128             # 6 projection K-chunks
URL_SPLIT = 25600            # url class A rows [0,25600), B rows [25600,50000)
ELEM = 128                   # gather row: 128 fp16 = 256 B
CH_T = 4                     # dst tiles per gather chunk (= 1 psum group)
N_CH = NT128 // CH_T         # 26 chunks
PB_LAG = 2                   # phase B trails phase A by this many chunks
KNOTS = (0.2, 0.6, 1.0, 1.4, 1.8)
N_LEFT = 3                   # knots 0..2 evaluated as left cubes relu(t-x)^3
XCLAMP = 2.2
GT_P = 34                    # gT partitions: 0-8 url agg, 9 ones, 32-33 sender

_LAST_RESULT = None
_CACHE = {}


# ----------------------------------------------------------------- host folds
def _head_weights(base_w, spline_w):
    """Returns (wh32 [128,6] f32 for [x,x^2,x^3],
                wh16 [128,12] f16 for [rsil, c1..c5], q0 [2]).
    Left-cube knots k<N_LEFT: relu(x-t)^3 = (x-t)^3 + relu(t-x)^3, the
    (x-t)^3 expands into the monomials."""
    c = np.array([1.0, -4.0, 6.0, -4.0, 1.0], np.float64)
    h = 0.4
    scale = 1.0 / (6.0 * h ** 3)
    O, D, B = spline_w.shape                      # [2, 128, 8]
    wp = np.zeros((O, D, 11), np.float64)         # W'[o,d,m], m=0..10
    for m in range(11):
        for j in range(5):
            b = m - j
            if 0 <= b < B:
                wp[:, :, m] += spline_w[:, :, b].astype(np.float64) * c[j] * scale
    t = np.arange(11) * h - 2.2                   # knot m at t_m
    q = np.zeros((4, O, D), np.float64)           # poly coeffs from m=0..5
    for m in range(6):
        q[0] += -t[m] ** 3 * wp[:, :, m]
        q[1] += 3 * t[m] ** 2 * wp[:, :, m]
        q[2] += -3 * t[m] * wp[:, :, m]
        q[3] += wp[:, :, m]
    # fold (x-t)^3 of the left-cube knots into the poly
    for k in range(N_LEFT):
        tk = KNOTS[k]
        w = wp[:, :, 6 + k]
        q[0] += -tk ** 3 * w
        q[1] += 3 * tk ** 2 * w
        q[2] += -3 * tk * w
        q[3] += w
    wh32 = np.zeros((D, 6), np.float64)
    for j in range(1, 4):                         # x, x^2, x^3
        wh32[:, 2 * (j - 1):2 * (j - 1) + 2] = q[j].T
    wh16 = np.zeros((D, 12), np.float64)
    wh16[:, 0:2] = base_w.T                       # rsil
    for k in range(5):                            # cube chunks
        wh16[:, 2 * (1 + k):2 * (1 + k) + 2] = wp[:, :, 6 + k].T
    q0 = q[0].sum(axis=1)                         # [2] constant, host-added
    return wh32.astype(np.float32), wh16.astype(NF16), q0.astype(np.float32)


def _fold_weights(inp):
    wrs = inp["w_root_se"] + inp["w_root_ue"]
    wbigT = (wrs @ inp["w_email"]).T.copy()                     # [768, 128]
    mc = np.zeros((GT_P, 128), np.float32)
    mc[0:8] = (inp["w_rel_ue"] @ inp["w_url"]).T
    mc[8] = inp["w_rel_ue"] @ inp["b_url"]
    mc[9] = inp["b_rel_se"] + inp["b_rel_ue"] + wrs @ inp["b_email"]
    mc[32] = inp["w_rel_se"] @ inp["w_sender"][:, 0]
    mc[33] = inp["w_rel_se"] @ inp["b_sender"]
    wh32, wh16, q0 = _head_weights(inp["base_w"], inp["spline_w"])
    return wbigT, mc, wh32, wh16, q0


def _wrap_idx16(flat):
    """int16 slot list -> [128, n/16] wrapped in 16 partitions, tiled to 128."""
    n = flat.shape[0]
    a = flat.astype(np.int16).reshape(n // 16, 16).T            # [16, n/16]
    return np.tile(a, (8, 1))


def _permute_nodes(dS):
    """Serpentine assignment of nodes (desc sender-degree) to NT128 tiles.
    Returns slot2node [NP] (-1 pads)."""
    order = np.argsort(-dS, kind="stable")
    n = len(order)
    i = np.arange(n)
    rnd, pos = i // NT128, i % NT128
    tiles = np.where(rnd % 2 == 0, pos, NT128 - 1 - pos)
    slot2node = np.full(NP, -1, np.int64)
    slot2node[tiles * 128 + rnd] = order
    return slot2node


def _prep_edges(inp):
    """Per-core per-class slot arrays (idx into class tables + local dst)."""
    cls_edges = []
    cls_edges.append((inp["se_src"], inp["se_dst"]))                     # S
    ua = inp["ue_src"] < URL_SPLIT
    cls_edges.append((inp["ue_src"][ua], inp["ue_dst"][ua]))             # A
    cls_edges.append((inp["ue_src"][~ua] - URL_SPLIT, inp["ue_dst"][~ua]))  # B
    zrow = (NS, URL_SPLIT, NU - URL_SPLIT)                               # zero-row ids

    s2n, n2s = [], []
    for c in range(N_CORES):
        sel = (inp["se_dst"] >= c * NSH) & (inp["se_dst"] < (c + 1) * NSH)
        dS = np.bincount(inp["se_dst"][sel] - c * NSH, minlength=NSH)
        sl = _permute_nodes(dS)
        inv = np.full(NSH, -1, np.int64)
        v = sl >= 0
        inv[sl[v]] = np.nonzero(v)[0]
        assert (inv >= 0).all()
        s2n.append(sl)
        n2s.append(inv)

    percls = []
    for ci, (src, dst) in enumerate(cls_edges):
        per_core = []
        gmax = 1
        for c in range(N_CORES):
            sel = (dst >= c * NSH) & (dst < (c + 1) * NSH)
            s, d = src[sel], n2s[c][dst[sel] - c * NSH]   # d = slot id
            order = np.argsort(d, kind="stable")
            s, d = s[order], d[order]
            t = d // 128
            cnt = np.bincount(t, minlength=NT128)
            gmax = max(gmax, int(np.ceil(cnt.max() / 128)))
            per_core.append((s, d, t, cnt))
        percls.append((per_core, gmax, zrow[ci]))

    out = []
    for per_core, g, zr in percls:
        idxs, dsts = [], []
        cap = g * 128
        for c in range(N_CORES):
            s, d, t, cnt = per_core[c]
            slots = np.full((NT128, cap), zr, np.int32)
            dloc = np.full((NT128, cap), -1, np.int32)
            start = np.zeros(NT128 + 1, np.int64)
            np.cumsum(cnt, out=start[1:])
            pos = np.arange(len(d)) - start[t]
            slots[t, pos] = s
            dloc[t, pos] = d % 128
            flat = slots.reshape(-1)
            idxs.append(_wrap_idx16(flat))
            oh = (dloc.reshape(NT128 * g, 128).T[:, :, None]
                  == np.arange(128)[None, None, :]).astype(NF16)
            dsts.append(oh.reshape(128, NT128 * g * 128))
        out.append((np.stack(idxs), np.stack(dsts), g))
    return out, s2n


# ----------------------------------------------------------------- device build
def _build(gS, gA, gB):
    nc = bacc.Bacc("TRN2", target_bir_lowering=False, debug=False,
                   num_devices=N_CORES, num_swdge_queues=4)
    dt = lambda n, s, d, k: nc.dram_tensor(n, s, d, kind=k).ap()
    xT = dt("xT", [KIN, NP], F16, "ExternalInput")
    tabS = dt("tabS", [NS + 1, ELEM], F16, "ExternalInput")
    tabA = dt("tabA", [URL_SPLIT + 1, ELEM], F16, "ExternalInput")
    tabB = dt("tabB", [NU - URL_SPLIT + 1, ELEM], F16, "ExternalInput")
    idxS = dt("idxS", [128, NT128 * gS * 8], mybir.dt.int16, "ExternalInput")
    idxA = dt("idxA", [128, NT128 * gA * 8], mybir.dt.int16, "ExternalInput")
    idxB = dt("idxB", [128, NT128 * gB * 8], mybir.dt.int16, "ExternalInput")
    ohS = dt("ohS", [128, NT128 * gS * 128], F16, "ExternalInput")
    ohA = dt("ohA", [128, NT128 * gA * 128], F16, "ExternalInput")
    ohB = dt("ohB", [128, NT128 * gB * 128], F16, "ExternalInput")
    wbigT = dt("wbigT", [KIN, HID], F16, "ExternalInput")
    mcomb = dt("mcomb", [GT_P, HID], F16, "ExternalInput")
    whead32 = dt("whead32", [HID, 6], F32, "ExternalInput")
    whead16 = dt("whead16", [HID, 12], F16, "ExternalInput")
    fillD = dt("fillD", [23, NP], F16, "ExternalInput")
    outT = dt("outT", [66, NP], F32, "ExternalOutput")

    with tile.TileContext(nc) as tc:
        import contextlib
        with contextlib.ExitStack() as ctx:
            persist = ctx.enter_context(tc.tile_pool(name="persist", bufs=1))
            gpool = ctx.enter_context(tc.tile_pool(name="gath", bufs=6))
            opool = ctx.enter_context(tc.tile_pool(name="oh", bufs=6))
            xpool = ctx.enter_context(tc.tile_pool(name="x", bufs=3))
            ew = ctx.enter_context(tc.tile_pool(name="ew", bufs=2))
            psum = ctx.enter_context(tc.tile_pool(name="ps", bufs=2, space="PSUM"))

            # ---- persistent small tensors
            gT = persist.tile([GT_P, NP], F16)
            # row 9 = ones (const term), rows 10-31 = zeros (feed zero mc rows)
            nc.sync.dma_start(out=gT[9:32, :], in_=fillD[:, :])
            wb = persist.tile([128, NKC * HID], F16)
            nc.sync.dma_start(
                out=wb[:].rearrange("p (c h) -> p c h", c=NKC),
                in_=wbigT.rearrange("(c p) h -> p c h", p=128))
            mcA = persist.tile([GT_P, HID], F16)
            nc.sync.dma_start(out=mcA[:], in_=mcomb[:, :])
            wh32 = persist.tile([HID, 6], F32)
            nc.sync.dma_start(out=wh32[:], in_=whead32[:])
            wh16 = persist.tile([HID, 12], F16)
            nc.sync.dma_start(out=wh16[:], in_=whead16[:])
            isbS = persist.tile([128, NT128 * gS * 8], mybir.dt.int16)
            nc.sync.dma_start(out=isbS[:], in_=idxS[:, :])
            isbA = persist.tile([128, NT128 * gA * 8], mybir.dt.int16)
            nc.sync.dma_start(out=isbA[:], in_=idxA[:, :])
            isbB = persist.tile([128, NT128 * gB * 8], mybir.dt.int16)
            nc.sync.dma_start(out=isbB[:], in_=idxB[:, :])
            kbias = persist.tile([128, len(KNOTS)], F32)
            for k, tk in enumerate(KNOTS):
                # left cubes use relu(t - x) = relu(-x + t); right relu(x - t)
                nc.gpsimd.memset(kbias[:, k:k + 1], tk if k < N_LEFT else -tk)

            # ---- phase B emitter (one 512-node tile)
            def phase_b(nt):
                ns = slice(nt * 512, (nt + 1) * 512)
                xs = xpool.tile([128, NKC * 512], F16, tag="xs")
                nc.sync.dma_start(
                    out=xs[:].rearrange("p (c n) -> p c n", c=NKC),
                    in_=xT[:, ns].rearrange("(c p) n -> p c n", p=128))
                pP = psum.tile([128, 512], F32, space="PSUM", tag="pP")
                for k in range(NKC):
                    nc.tensor.matmul(
                        out=pP[:], lhsT=wb[:, k * HID:(k + 1) * HID],
                        rhs=xs[:, k * 512:(k + 1) * 512],
                        start=(k == 0), stop=False)
                nc.tensor.matmul(out=pP[:], lhsT=mcA[:], rhs=gT[:, ns],
                                 start=False, stop=True)

                xt = ew.tile([128, 512], F32, tag="xt")      # clamp(h,0,2.2)
                nc.vector.tensor_scalar(out=xt[:], in0=pP[:], scalar1=0.0,
                                        scalar2=XCLAMP,
                                        op0=mybir.AluOpType.max,
                                        op1=mybir.AluOpType.min)
                sil = ew.tile([128, 512], F16, tag="sil")
                nc.scalar.activation(sil[:], pP[:],
                                     mybir.ActivationFunctionType.Silu)
                rsil = ew.tile([128, 512], F16, tag="rsil")
                nc.vector.tensor_scalar_max(rsil[:], sil[:], 0.0)
                x2 = ew.tile([128, 512], F32, tag="x2")
                nc.scalar.square(x2[:], xt[:])
                x3 = ew.tile([128, 512], F32, tag="x3")
                nc.vector.tensor_tensor(out=x3[:], in0=x2[:], in1=xt[:],
                                        op=mybir.AluOpType.mult)
                cubes = []
                for k in range(5):
                    r = ew.tile([128, 512], F16, tag=f"r{k}")
                    nc.scalar.activation(r[:], xt[:],
                                         mybir.ActivationFunctionType.Relu,
                                         bias=kbias[:, k:k + 1],
                                         scale=(-1.0 if k < N_LEFT else 1.0))
                    sq = ew.tile([128, 512], F16, tag=f"sq{k}")
                    nc.vector.tensor_tensor(out=sq[:], in0=r[:], in1=r[:],
                                            op=mybir.AluOpType.mult)
                    r3 = ew.tile([128, 512], F16, tag=f"rrr{k}")
                    nc.vector.tensor_tensor(out=r3[:], in0=sq[:], in1=r[:],
                                            op=mybir.AluOpType.mult)
                    cubes.append(r3)
                # 9 chunks, 3-way column tiling; strip chains:
                #  s0: x, rsil, c3   s32: x^2, c1, c4   s64: x^3, c2, c5
                pO = psum.tile([66, 512], F32, space="PSUM", tag="pO")
                for j, ck in enumerate([xt, x2, x3]):
                    nc.tensor.matmul(out=pO[32 * j:32 * j + 2, :],
                                     lhsT=wh32[:, 2 * j:2 * j + 2],
                                     rhs=ck[:], start=True, stop=False)
                for j, ck in enumerate([rsil] + cubes):
                    s = 32 * (j % 3)
                    nc.tensor.matmul(out=pO[s:s + 2, :],
                                     lhsT=wh16[:, 2 * j:2 * j + 2],
                                     rhs=ck[:], start=False, stop=(j >= 3))
                ot = ew.tile([66, 512], F32, tag="ot")
                nc.scalar.copy(out=ot[:], in_=pO[0:66, :])
                nc.sync.dma_start(out=outT[:, ns], in_=ot[:])

            # ---- phase A: gather + one-hot scatter matmuls
            for ch in range(N_CH):
                t0 = ch * CH_T
                cls = []
                for qn, (tab, isb, g, ohd) in enumerate(
                        ((tabS, isbS, gS, ohS),
                         (tabA, isbA, gA, ohA),
                         (tabB, isbB, gB, ohB))):
                    nidx = CH_T * g * 128
                    gt = gpool.tile([128, CH_T * g, ELEM], F16,
                                    tag=f"g{qn}")
                    nc.gpsimd.dma_gather(
                        out_ap=gt[:], in_ap=tab[:],
                        idxs_ap=isb[:, t0 * g * 8:(t0 + CH_T) * g * 8],
                        num_idxs=nidx, num_idxs_reg=nidx, elem_size=ELEM,
                        single_packet=False, queue_num=(ch * 3 + qn) % 4)
                    ohsb = opool.tile([128, CH_T * g * 128], F16,
                                      tag=f"oh{qn}")
                    nc.sync.dma_start(
                        out=ohsb[:],
                        in_=ohd[:, t0 * g * 128:(t0 + CH_T) * g * 128])
                    cls.append((gt, g, ohsb))
                pSU = psum.tile([GT_P, 512], F32, space="PSUM", tag="pSU")
                for tl in range(CH_T):
                    csl = slice(tl * 128, (tl + 1) * 128)
                    for ci, (gt, g, ohsb) in enumerate(cls):
                        for gi in range(g):
                            gcol = tl * g + gi
                            oh = ohsb[:, gcol * 128:(gcol + 1) * 128]
                            if ci == 0:      # sender -> strip at partition 32
                                nc.tensor.matmul(
                                    out=pSU[32:34, csl],
                                    lhsT=gt[:, gcol, 0:2], rhs=oh,
                                    start=(gi == 0), stop=(gi == gS - 1))
                            else:            # urls -> partitions 0-8
                                nc.tensor.matmul(
                                    out=pSU[0:9, csl],
                                    lhsT=gt[:, gcol, 0:9], rhs=oh,
                                    start=(ci == 1 and gi == 0),
                                    stop=(ci == 2 and gi == gB - 1))
                gsl = slice(ch * 512, (ch + 1) * 512)
                nc.scalar.copy(out=gT[0:9, gsl], in_=pSU[0:9, :])
                nc.vector.tensor_copy(out=gT[32:34, gsl], in_=pSU[32:34, :])
                # phase B lags phase A so its psum/DVE chain never waits on
                # the gather pipeline of the same chunk
                if ch >= PB_LAG:
                    phase_b(ch - PB_LAG)
            for nt in range(N_CH - PB_LAG, N_CH):
                phase_b(nt)

    nc.compile()
    return nc


# ----------------------------------------------------------------- entry point
def kernel(**inp):
    inp = {k: np.asarray(v) for k, v in inp.items()}
    wbigT, mcomb, wh32, wh16, q0 = _fold_weights(inp)
    eprep, s2n = _prep_edges(inp)
    (idxS, ohS_, gS), (idxA, ohA_, gA), (idxB, ohB_, gB) = eprep

    key = (gS, gA, gB)
    if key not in _CACHE:
        _CACHE[key] = _build(gS, gA, gB)
    nc = _CACHE[key]

    tabS = np.zeros((NS + 1, ELEM), NF16)
    tabS[:NS, 0] = inp["x_sender"][:, 0].astype(NF16)
    tabS[:NS, 1] = 1
    tabA = np.zeros((URL_SPLIT + 1, ELEM), NF16)
    tabA[:URL_SPLIT, 0:8] = inp["x_url"][:URL_SPLIT].astype(NF16)
    tabA[:URL_SPLIT, 8] = 1
    tabB = np.zeros((NU - URL_SPLIT + 1, ELEM), NF16)
    tabB[: NU - URL_SPLIT, 0:8] = inp["x_url"][URL_SPLIT:].astype(NF16)
    tabB[: NU - URL_SPLIT, 8] = 1

    fill = np.zeros((23, NP), NF16)
    fill[0, :] = 1
    in_maps = []
    for c in range(N_CORES):
        xsh = np.zeros((KIN, NP), NF16)
        sl = s2n[c]
        v = sl >= 0
        xsh[:, v] = inp["x_email"][c * NSH + sl[v]].T.astype(NF16)
        in_maps.append({
            "xT": xsh, "tabS": tabS, "tabA": tabA, "tabB": tabB,
            "idxS": idxS[c], "idxA": idxA[c], "idxB": idxB[c],
            "ohS": ohS_[c], "ohA": ohA_[c], "ohB": ohB_[c],
            "wbigT": wbigT.astype(NF16), "mcomb": mcomb.astype(NF16),
            "whead32": wh32, "whead16": wh16, "fillD": fill,
        })

    global _LAST_RESULT
    trace = os.environ.get("KERNEL_TRACE", "0") == "1"
    kw = {}
    td = os.environ.get("KERNEL_TMPDIR")
    if td:
        os.makedirs(td, exist_ok=True)
        kw["tmpdir"] = td
    res = run_bass_kernel_spmd(nc, in_maps, core_ids=list(range(N_CORES)),
                               trace=trace, **kw)
    _LAST_RESULT = res
    out = np.empty((NE, 2), np.float32)
    for c in range(N_CORES):
        o66 = res.results[c]["outT"]
        o = (o66[0:2] + o66[32:34] + o66[64:66]) + q0[:, None]
        sl = s2n[c]
        v = sl >= 0
        out[c * NSH + sl[v]] = o[:, v].T
    return out


# revision 21
# speedup vs baseline: 1.0992x; 1.0992x over previous
"""Trainium2 Bass kernel for nn_HKANGNN (hetero GraphConv + KAN head).

Math (only the email-node output path matters):
  e    = x_email @ w_email.T + b_email
  agg_se[n] = sum_{se edges -> n} (x_sender[src] @ w_sender.T + b_sender)
  agg_ue[n] = sum_{ue edges -> n} (x_url[src]    @ w_url.T    + b_url)
  out_e = agg_se @ w_rel_se.T + b_rel_se + agg_ue @ w_rel_ue.T + b_rel_ue
        + e @ (w_root_se + w_root_ue).T
  h = relu(out_e);  out = silu(h) @ base_w.T + einsum(b_splines(h), spline_w)

Device strategy (8 cores, email nodes sharded 12500/core, padded to 13312):
  * linearity folds the tiny projections into a single [34,128] mc_all matmul
    over per-dst aggregates of RAW features + counts (all fp16).
  * per-core node permutation (serpentine by sender-degree over 104 dst
    tiles) balances per-tile edge counts: sender class needs 1 gather group
    per tile, urls 2 -> 66560 gather slots (vs 89600 naive).
  * segment-sum = one-hot matmuls accumulated in a [*,512] PSUM covering 4
    dst tiles; per-edge rows via dma_gather on 4 SWDGE queues.
  * projection: out_e.T[h, n] accumulated in PSUM over 6 K-chunks of
    (Wrootsum@w_email).T as stationary weights (fp16).
  * KAN head with x = clamp(h,0,2.2): spline(h) = poly3(x) [f32 chunks]
    + sum_k w_k * cube_k where cube_k = relu(t_k-x)^3 for t_k<=1.0 (left
    cubes; the (x-t)^3 part folds into the poly) and relu(x-t_k)^3 for
    t_k>1.0.  The cube/rsil chunks are small-valued -> fp16-safe (2x DVE),
    only the monomials stay f32.  9 chunks run 3-way column-tiled on the
    PE; host sums the 3 output strips + q0 during unsharding.
"""

import os
import numpy as np

import concourse.bass as bass
import concourse.mybir as mybir
import concourse.tile as tile
from concourse import bacc
from concourse.bass_utils import run_bass_kernel_spmd

F32 = mybir.dt.float32
F16 = mybir.dt.float16
NF16 = np.float16

N_CORES = 8
HID = 128
NE, NS, NU = 100000, 30000, 50000
NSH = NE // N_CORES          # 12500 real nodes per core
NP = 13312                   # padded (26 x 512 node tiles, 104 x 128 dst tiles)
NT128 = NP // 128            # 104 dst tiles
NT512 = NP // 512            # 26 node tiles
KIN = 768
NKC = KIN // 128             # 6 projection K-chunks
URL_SPLIT = 25600            # url class A rows [0,25600), B rows [25600,50000)
ELEM = 128                   # gather row: 128 fp16 = 256 B
CH_T = 4                     # dst tiles per gather chunk (= 1 psum group)
N_CH = NT128 // CH_T         # 26 chunks
PB_LAG = 2                   # phase B trails phase A by this many chunks
KNOTS = (0.2, 0.6, 1.0, 1.4, 1.8)
N_LEFT = 3                   # knots 0..2 evaluated as left cubes relu(t-x)^3
XCLAMP = 2.2
GT_P = 34                    # gT partitions: 0-8 url agg, 9 ones, 32-33 sender

_LAST_RESULT = None
_CACHE = {}


# ----------------------------------------------------------------- host folds
def _head_weights(base_w, spline_w):
    """Returns (wh32 [128,6] f32 for [x,x^2,x^3],
                wh16 [128,12] f16 for [rsil, c1..c5], q0 [2]).
    Left-cube knots k<N_LEFT: relu(x-t)^3 = (x-t)^3 + relu(t-x)^3, the
    (x-t)^3 expands into the monomials."""
    c = np.array([1.0, -4.0, 6.0, -4.0, 1.0], np.float64)
    h = 0.4
    scale = 1.0 / (6.0 * h ** 3)
    O, D, B = spline_w.shape                      # [2, 128, 8]
    wp = np.zeros((O, D, 11), np.float64)         # W'[o,d,m], m=0..10
    for m in range(11):
        for j in range(5):
            b = m - j
            if 0 <= b < B:
                wp[:, :, m] += spline_w[:, :, b].astype(np.float64) * c[j] * scale
    t = np.arange(11) * h - 2.2                   # knot m at t_m
    q = np.zeros((4, O, D), np.float64)           # poly coeffs from m=0..5
    for m in range(6):
        q[0] += -t[m] ** 3 * wp[:, :, m]
        q[1] += 3 * t[m] ** 2 * wp[:, :, m]
        q[2] += -3 * t[m] * wp[:, :, m]
        q[3] += wp[:, :, m]
    # fold (x-t)^3 of the left-cube knots into the poly
    for k in range(N_LEFT):
        tk = KNOTS[k]
        w = wp[:, :, 6 + k]
        q[0] += -tk ** 3 * w
        q[1] += 3 * tk ** 2 * w
        q[2] += -3 * tk * w
        q[3] += w
    wh32 = np.zeros((D, 6), np.float64)
    for j in range(1, 4):                         # x, x^2, x^3
        wh32[:, 2 * (j - 1):2 * (j - 1) + 2] = q[j].T
    wh16 = np.zeros((D, 12), np.float64)
    wh16[:, 0:2] = base_w.T                       # rsil
    for k in range(5):                            # cube chunks
        wh16[:, 2 * (1 + k):2 * (1 + k) + 2] = wp[:, :, 6 + k].T
    q0 = q[0].sum(axis=1)                         # [2] constant, host-added
    return wh32.astype(np.float32), wh16.astype(NF16), q0.astype(np.float32)


def _fold_weights(inp):
    wrs = inp["w_root_se"] + inp["w_root_ue"]
    wbigT = (wrs @ inp["w_email"]).T.copy()                     # [768, 128]
    mc = np.zeros((GT_P, 128), np.float32)
    mc[0:8] = (inp["w_rel_ue"] @ inp["w_url"]).T
    mc[8] = inp["w_rel_ue"] @ inp["b_url"]
    mc[9] = inp["b_rel_se"] + inp["b_rel_ue"] + wrs @ inp["b_email"]
    mc[32] = inp["w_rel_se"] @ inp["w_sender"][:, 0]
    mc[33] = inp["w_rel_se"] @ inp["b_sender"]
    wh32, wh16, q0 = _head_weights(inp["base_w"], inp["spline_w"])
    return wbigT, mc, wh32, wh16, q0


def _wrap_idx16(flat):
    """int16 slot list -> [128, n/16] wrapped in 16 partitions, tiled to 128."""
    n = flat.shape[0]
    a = flat.astype(np.int16).reshape(n // 16, 16).T            # [16, n/16]
    return np.tile(a, (8, 1))


def _permute_nodes(dS):
    """Serpentine assignment of nodes (desc sender-degree) to NT128 tiles.
    Returns slot2node [NP] (-1 pads)."""
    order = np.argsort(-dS, kind="stable")
    n = len(order)
    i = np.arange(n)
    rnd, pos = i // NT128, i % NT128
    tiles = np.where(rnd % 2 == 0, pos, NT128 - 1 - pos)
    slot2node = np.full(NP, -1, np.int64)
    slot2node[tiles * 128 + rnd] = order
    return slot2node


def _prep_edges(inp):
    """Per-core per-class slot arrays (idx into class tables + local dst)."""
    cls_edges = []
    cls_edges.append((inp["se_src"], inp["se_dst"]))                     # S
    ua = inp["ue_src"] < URL_SPLIT
    cls_edges.append((inp["ue_src"][ua], inp["ue_dst"][ua]))             # A
    cls_edges.append((inp["ue_src"][~ua] - URL_SPLIT, inp["ue_dst"][~ua]))  # B
    zrow = (NS, URL_SPLIT, NU - URL_SPLIT)                               # zero-row ids

    s2n, n2s = [], []
    for c in range(N_CORES):
        sel = (inp["se_dst"] >= c * NSH) & (inp["se_dst"] < (c + 1) * NSH)
        dS = np.bincount(inp["se_dst"][sel] - c * NSH, minlength=NSH)
        sl = _permute_nodes(dS)
        inv = np.full(NSH, -1, np.int64)
        v = sl >= 0
        inv[sl[v]] = np.nonzero(v)[0]
        assert (inv >= 0).all()
        s2n.append(sl)
        n2s.append(inv)

    percls = []
    for ci, (src, dst) in enumerate(cls_edges):
        per_core = []
        gmax = 1
        for c in range(N_CORES):
            sel = (dst >= c * NSH) & (dst < (c + 1) * NSH)
            s, d = src[sel], n2s[c][dst[sel] - c * NSH]   # d = slot id
            order = np.argsort(d, kind="stable")
            s, d = s[order], d[order]
            t = d // 128
            cnt = np.bincount(t, minlength=NT128)
            gmax = max(gmax, int(np.ceil(cnt.max() / 128)))
            per_core.append((s, d, t, cnt))
        percls.append((per_core, gmax, zrow[ci]))

    out = []
    for per_core, g, zr in percls:
        idxs, dsts = [], []
        cap = g * 128
        for c in range(N_CORES):
            s, d, t, cnt = per_core[c]
            slots = np.full((NT128, cap), zr, np.int32)
            dloc = np.full((NT128, cap), -1, np.int32)
            start = np.zeros(NT128 + 1, np.int64)
            np.cumsum(cnt, out=start[1:])
            pos = np.arange(len(d)) - start[t]
            slots[t, pos] = s
            dloc[t, pos] = d % 128
            flat = slots.reshape(-1)
            idxs.append(_wrap_idx16(flat))
            oh = (dloc.reshape(NT128 * g, 128).T[:, :, None]
                  == np.arange(128)[None, None, :]).astype(NF16)
            dsts.append(oh.reshape(128, NT128 * g * 128))
        out.append((np.stack(idxs), np.stack(dsts), g))
    return out, s2n


# ----------------------------------------------------------------- device build
def _build(gS, gA, gB):
    nc = bacc.Bacc("TRN2", target_bir_lowering=False, debug=False,
                   num_devices=N_CORES, num_swdge_queues=4)
    dt = lambda n, s, d, k: nc.dram_tensor(n, s, d, kind=k).ap()
    xT = dt("xT", [KIN, NP], F16, "ExternalInput")
    tabS = dt("tabS", [NS + 1, ELEM], F16, "ExternalInput")
    tabA = dt("tabA", [URL_SPLIT + 1, ELEM], F16, "ExternalInput")
    tabB = dt("tabB", [NU - URL_SPLIT + 1, ELEM], F16, "ExternalInput")
    idxS = dt("idxS", [128, NT128 * gS * 8], mybir.dt.int16, "ExternalInput")
    idxA = dt("idxA", [128, NT128 * gA * 8], mybir.dt.int16, "ExternalInput")
    idxB = dt("idxB", [128, NT128 * gB * 8], mybir.dt.int16, "ExternalInput")
    ohS = dt("ohS", [128, NT128 * gS * 128], F16, "ExternalInput")
    ohA = dt("ohA", [128, NT128 * gA * 128], F16, "ExternalInput")
    ohB = dt("ohB", [128, NT128 * gB * 128], F16, "ExternalInput")
    wbigT = dt("wbigT", [KIN, HID], F16, "ExternalInput")
    mcomb = dt("mcomb", [GT_P, HID], F16, "ExternalInput")
    whead32 = dt("whead32", [HID, 6], F32, "ExternalInput")
    whead16 = dt("whead16", [HID, 12], F16, "ExternalInput")
    fillD = dt("fillD", [23, NP], F16, "ExternalInput")
    outT = dt("outT", [66, NP], F32, "ExternalOutput")

    with tile.TileContext(nc) as tc:
        import contextlib
        with contextlib.ExitStack() as ctx:
            persist = ctx.enter_context(tc.tile_pool(name="persist", bufs=1))
            gpool = ctx.enter_context(tc.tile_pool(name="gath", bufs=6))
            opool = ctx.enter_context(tc.tile_pool(name="oh", bufs=6))
            xpool = ctx.enter_context(tc.tile_pool(name="x", bufs=3))
            ew = ctx.enter_context(tc.tile_pool(name="ew", bufs=3))
            psum = ctx.enter_context(tc.tile_pool(name="ps", bufs=2, space="PSUM"))

            # ---- persistent small tensors
            gT = persist.tile([GT_P, NP], F16)
            # row 9 = ones (const term), rows 10-31 = zeros (feed zero mc rows)
            nc.sync.dma_start(out=gT[9:32, :], in_=fillD[:, :])
            wb = persist.tile([128, NKC * HID], F16)
            nc.sync.dma_start(
                out=wb[:].rearrange("p (c h) -> p c h", c=NKC),
                in_=wbigT.rearrange("(c p) h -> p c h", p=128))
            mcA = persist.tile([GT_P, HID], F16)
            nc.sync.dma_start(out=mcA[:], in_=mcomb[:, :])
            wh32 = persist.tile([HID, 6], F32)
            nc.sync.dma_start(out=wh32[:], in_=whead32[:])
            wh16 = persist.tile([HID, 12], F16)
            nc.sync.dma_start(out=wh16[:], in_=whead16[:])
            isbS = persist.tile([128, NT128 * gS * 8], mybir.dt.int16)
            nc.sync.dma_start(out=isbS[:], in_=idxS[:, :])
            isbA = persist.tile([128, NT128 * gA * 8], mybir.dt.int16)
            nc.sync.dma_start(out=isbA[:], in_=idxA[:, :])
            isbB = persist.tile([128, NT128 * gB * 8], mybir.dt.int16)
            nc.sync.dma_start(out=isbB[:], in_=idxB[:, :])
            kbias = persist.tile([128, len(KNOTS)], F32)
            for k, tk in enumerate(KNOTS):
                # left cubes use relu(t - x) = relu(-x + t); right relu(x - t)
                nc.gpsimd.memset(kbias[:, k:k + 1], tk if k < N_LEFT else -tk)

            # ---- phase B emitter (one 512-node tile)
            def phase_b(nt):
                ns = slice(nt * 512, (nt + 1) * 512)
                xs = xpool.tile([128, NKC * 512], F16, tag="xs")
                nc.sync.dma_start(
                    out=xs[:].rearrange("p (c n) -> p c n", c=NKC),
                    in_=xT[:, ns].rearrange("(c p) n -> p c n", p=128))
                pP = psum.tile([128, 512], F32, space="PSUM", tag="pP", bufs=3)
                for k in range(NKC):
                    nc.tensor.matmul(
                        out=pP[:], lhsT=wb[:, k * HID:(k + 1) * HID],
                        rhs=xs[:, k * 512:(k + 1) * 512],
                        start=(k == 0), stop=False)
                nc.tensor.matmul(out=pP[:], lhsT=mcA[:], rhs=gT[:, ns],
                                 start=False, stop=True)

                xt = ew.tile([128, 512], F32, tag="xt")      # clamp(h,0,2.2)
                nc.vector.tensor_scalar(out=xt[:], in0=pP[:], scalar1=0.0,
                                        scalar2=XCLAMP,
                                        op0=mybir.AluOpType.max,
                                        op1=mybir.AluOpType.min)
                sil = ew.tile([128, 512], F16, tag="sil")
                nc.scalar.activation(sil[:], pP[:],
                                     mybir.ActivationFunctionType.Silu)
                rsil = ew.tile([128, 512], F16, tag="rsil")
                nc.vector.tensor_scalar_max(rsil[:], sil[:], 0.0)
                x2 = ew.tile([128, 512], F32, tag="x2")
                nc.scalar.square(x2[:], xt[:])
                x3 = ew.tile([128, 512], F32, tag="x3")
                nc.vector.tensor_tensor(out=x3[:], in0=x2[:], in1=xt[:],
                                        op=mybir.AluOpType.mult)
                cubes = []
                for k in range(5):
                    r = ew.tile([128, 512], F16, tag=f"r{k}")
                    nc.scalar.activation(r[:], xt[:],
                                         mybir.ActivationFunctionType.Relu,
                                         bias=kbias[:, k:k + 1],
                                         scale=(-1.0 if k < N_LEFT else 1.0))
                    sq = ew.tile([128, 512], F16, tag=f"sq{k}")
                    nc.vector.tensor_tensor(out=sq[:], in0=r[:], in1=r[:],
                                            op=mybir.AluOpType.mult)
                    r3 = ew.tile([128, 512], F16, tag=f"rrr{k}")
                    nc.vector.tensor_tensor(out=r3[:], in0=sq[:], in1=r[:],
                                            op=mybir.AluOpType.mult)
                    cubes.append(r3)
                # 9 chunks, 3-way column tiling; strip chains:
                #  s0: x, rsil, c3   s32: x^2, c1, c4   s64: x^3, c2, c5
                pO = psum.tile([66, 512], F32, space="PSUM", tag="pO", bufs=3)
                for j, ck in enumerate([xt, x2, x3]):
                    nc.tensor.matmul(out=pO[32 * j:32 * j + 2, :],
                                     lhsT=wh32[:, 2 * j:2 * j + 2],
                                     rhs=ck[:], start=True, stop=False)
                for j, ck in enumerate([rsil] + cubes):
                    s = 32 * (j % 3)
                    nc.tensor.matmul(out=pO[s:s + 2, :],
                                     lhsT=wh16[:, 2 * j:2 * j + 2],
                                     rhs=ck[:], start=False, stop=(j >= 3))
                ot = ew.tile([66, 512], F32, tag="ot")
                nc.scalar.copy(out=ot[:], in_=pO[0:66, :])
                nc.sync.dma_start(out=outT[:, ns], in_=ot[:])

            # ---- phase A: gather + one-hot scatter matmuls
            for ch in range(N_CH):
                t0 = ch * CH_T
                cls = []
                for qn, (tab, isb, g, ohd) in enumerate(
                        ((tabS, isbS, gS, ohS),
                         (tabA, isbA, gA, ohA),
                         (tabB, isbB, gB, ohB))):
                    nidx = CH_T * g * 128
                    gt = gpool.tile([128, CH_T * g, ELEM], F16,
                                    tag=f"g{qn}")
                    nc.gpsimd.dma_gather(
                        out_ap=gt[:], in_ap=tab[:],
                        idxs_ap=isb[:, t0 * g * 8:(t0 + CH_T) * g * 8],
                        num_idxs=nidx, num_idxs_reg=nidx, elem_size=ELEM,
                        single_packet=False, queue_num=(ch * 3 + qn) % 4)
                    ohsb = opool.tile([128, CH_T * g * 128], F16,
                                      tag=f"oh{qn}")
                    nc.sync.dma_start(
                        out=ohsb[:],
                        in_=ohd[:, t0 * g * 128:(t0 + CH_T) * g * 128])
                    cls.append((gt, g, ohsb))
                # phase B first in emission order: its matmuls/DVE ops have
                # ready inputs, so they fill engine FIFOs ahead of the
                # scatter matmuls that wait on this chunk's gathers.
                if ch >= PB_LAG:
                    phase_b(ch - PB_LAG)
                pSU = psum.tile([GT_P, 512], F32, space="PSUM", tag="pSU")
                for tl in range(CH_T):
                    csl = slice(tl * 128, (tl + 1) * 128)
                    for ci, (gt, g, ohsb) in enumerate(cls):
                        for gi in range(g):
                            gcol = tl * g + gi
                            oh = ohsb[:, gcol * 128:(gcol + 1) * 128]
                            if ci == 0:      # sender -> strip at partition 32
                                nc.tensor.matmul(
                                    out=pSU[32:34, csl],
                                    lhsT=gt[:, gcol, 0:2], rhs=oh,
                                    start=(gi == 0), stop=(gi == gS - 1))
                            else:            # urls -> partitions 0-8
                                nc.tensor.matmul(
                                    out=pSU[0:9, csl],
                                    lhsT=gt[:, gcol, 0:9], rhs=oh,
                                    start=(ci == 1 and gi == 0),
                                    stop=(ci == 2 and gi == gB - 1))
                gsl = slice(ch * 512, (ch + 1) * 512)
                nc.scalar.copy(out=gT[0:9, gsl], in_=pSU[0:9, :])
                nc.vector.tensor_copy(out=gT[32:34, gsl], in_=pSU[32:34, :])
            for nt in range(N_CH - PB_LAG, N_CH):
                phase_b(nt)

    nc.compile()
    return nc


# ----------------------------------------------------------------- entry point
def kernel(**inp):
    inp = {k: np.asarray(v) for k, v in inp.items()}
    wbigT, mcomb, wh32, wh16, q0 = _fold_weights(inp)
    eprep, s2n = _prep_edges(inp)
    (idxS, ohS_, gS), (idxA, ohA_, gA), (idxB, ohB_, gB) = eprep

    key = (gS, gA, gB)
    if key not in _CACHE:
        _CACHE[key] = _build(gS, gA, gB)
    nc = _CACHE[key]

    tabS = np.zeros((NS + 1, ELEM), NF16)
    tabS[:NS, 0] = inp["x_sender"][:, 0].astype(NF16)
    tabS[:NS, 1] = 1
    tabA = np.zeros((URL_SPLIT + 1, ELEM), NF16)
    tabA[:URL_SPLIT, 0:8] = inp["x_url"][:URL_SPLIT].astype(NF16)
    tabA[:URL_SPLIT, 8] = 1
    tabB = np.zeros((NU - URL_SPLIT + 1, ELEM), NF16)
    tabB[: NU - URL_SPLIT, 0:8] = inp["x_url"][URL_SPLIT:].astype(NF16)
    tabB[: NU - URL_SPLIT, 8] = 1

    fill = np.zeros((23, NP), NF16)
    fill[0, :] = 1
    in_maps = []
    for c in range(N_CORES):
        xsh = np.zeros((KIN, NP), NF16)
        sl = s2n[c]
        v = sl >= 0
        xsh[:, v] = inp["x_email"][c * NSH + sl[v]].T.astype(NF16)
        in_maps.append({
            "xT": xsh, "tabS": tabS, "tabA": tabA, "tabB": tabB,
            "idxS": idxS[c], "idxA": idxA[c], "idxB": idxB[c],
            "ohS": ohS_[c], "ohA": ohA_[c], "ohB": ohB_[c],
            "wbigT": wbigT.astype(NF16), "mcomb": mcomb.astype(NF16),
            "whead32": wh32, "whead16": wh16, "fillD": fill,
        })

    global _LAST_RESULT
    trace = os.environ.get("KERNEL_TRACE", "0") == "1"
    kw = {}
    td = os.environ.get("KERNEL_TMPDIR")
    if td:
        os.makedirs(td, exist_ok=True)
        kw["tmpdir"] = td
    res = run_bass_kernel_spmd(nc, in_maps, core_ids=list(range(N_CORES)),
                               trace=trace, **kw)
    _LAST_RESULT = res
    out = np.empty((NE, 2), np.float32)
    for c in range(N_CORES):
        o66 = res.results[c]["outT"]
        o = (o66[0:2] + o66[32:34] + o66[64:66]) + q0[:, None]
        sl = s2n[c]
        v = sl >= 0
        out[c * NSH + sl[v]] = o[:, v].T
    return out
